# revision 9
# baseline (speedup 1.0000x reference)
"""Trainium2 Bass kernel for ExponentialConcordanceLoss.

Reference semantics (N = 8192):
    t = targets[:, 0]; e = targets[:, 1] != 0; s = preds
    mask[j, i] = (t[i] < t[j]) & e[i]            (all inputs finite)
    loss = sum_{j,i} mask * exp(s[j] - s[i]) / max(sum(mask), 1)

Factorization used on device:
    loss_sum = sum_j exp(s[j]) * (sum_i mask[j,i] * exp(-s[i]))
    count    = sum_{j,i} mask[j,i]

Sharding: rows j are split across 8 cores (1024 rows each, 8 blocks of
128 partitions). Each core holds the full i-axis (8192) in SBUF.
The event filter is folded into the compare by t'_i = t_i + 1e30*(e_i==0),
so per 128-row block only two full-width DVE ops are needed:
  pass1: m[j,i] = (t'_i < t_j)              tensor_scalar, fused row-count
  pass2: sum_i m[j,i] * exp(-s_i)           tensor_tensor_reduce
The per-row results are scaled by exp(s_j) and tree-reduced on device;
the host sums the 8 per-core (loss_sum, count) pairs and divides.
"""

import sys

if "/opt/trn_rl_repo" not in sys.path:
    sys.path.insert(0, "/opt/trn_rl_repo")

import numpy as np

N = 8192
NCORES = 8
JPC = N // NCORES      # rows per core
NBLK = JPC // 128      # 128-row blocks per core
BCOL = N // 128        # compact layout columns

_CACHE = {}


def _build():
    """Trace the (SPMD) Bass program. Same program for every core; the
    per-core row slice arrives via the tj/sj input tensors."""
    import concourse.bass as bass
    import concourse.mybir as mybir

    f32 = mybir.dt.float32
    Alu = mybir.AluOpType
    Act = mybir.ActivationFunctionType
    X = mybir.AxisListType.X

    nc = bass.Bass()

    tcmp_d = nc.dram_tensor("tcmp", [128, BCOL], f32, kind="ExternalInput")
    ecmp_d = nc.dram_tensor("ecmp", [128, BCOL], f32, kind="ExternalInput")
    scmp_d = nc.dram_tensor("scmp", [128, BCOL], f32, kind="ExternalInput")
    tj_d = nc.dram_tensor("tj", [128, NBLK], f32, kind="ExternalInput")
    sj_d = nc.dram_tensor("sj", [128, NBLK], f32, kind="ExternalInput")
    out_d = nc.dram_tensor("out", [128, 2], f32, kind="ExternalOutput")
    scr1 = nc.dram_tensor("scr1", [N], f32, kind="Internal")
    scr2 = nc.dram_tensor("scr2", [N], f32, kind="Internal")

    # vector-op count checkpoints (vv semaphore values)
    VV_TEXC = 2                      # t' compact ready
    VV_MAIN = VV_TEXC + 2 * NBLK     # all pass1/pass2 done
    VV_DONE = VV_MAIN + 3            # epilogue done (mul, 2 reduces)

    with (
        nc.sbuf_tensor([128, BCOL], f32) as tcmp_s,
        nc.sbuf_tensor([128, BCOL], f32) as ecmp_s,
        nc.sbuf_tensor([128, BCOL], f32) as scmp_s,
        nc.sbuf_tensor([128, BCOL], f32) as tmp_s,
        nc.sbuf_tensor([128, BCOL], f32) as texc_s,
        nc.sbuf_tensor([128, BCOL], f32) as wcmp_s,
        nc.sbuf_tensor([128, NBLK], f32) as tj_s,
        nc.sbuf_tensor([128, NBLK], f32) as sj_s,
        nc.sbuf_tensor([128, NBLK], f32) as vj_s,
        nc.sbuf_tensor([128, NBLK], f32) as crows,
        nc.sbuf_tensor([128, NBLK], f32) as lrows,
        nc.sbuf_tensor([128, 2], f32) as red,
        nc.sbuf_tensor([128, N], f32) as texc_b,
        nc.sbuf_tensor([128, N], f32) as w_b,
        nc.sbuf_tensor([128, N], f32) as m_t,
        nc.sbuf_tensor([128, N], f32) as junk,
        nc.semaphore() as dsem,
        nc.semaphore() as dsem2,
        nc.semaphore() as asem,
        nc.semaphore() as vv,
        nc.Block() as block,
    ):
        @block.sync
        def _(sync):
            # five small input loads (cum dsem: 80)
            sync.dma_start(tcmp_s[:], tcmp_d[:]).then_inc(dsem, 16)
            sync.dma_start(ecmp_s[:], ecmp_d[:]).then_inc(dsem, 16)
            sync.dma_start(scmp_s[:], scmp_d[:]).then_inc(dsem, 16)
            sync.dma_start(tj_s[:], tj_d[:]).then_inc(dsem, 16)
            sync.dma_start(sj_s[:], sj_d[:]).then_inc(dsem, 16)
            # t' row -> DRAM (dense), then broadcast-read to all partitions
            sync.wait_ge(vv, VV_TEXC)
            sync.dma_start(
                scr1.rearrange("(p b) -> p b", b=BCOL), texc_s[:]
            ).then_inc(dsem, 16)  # cum 96
            sync.wait_ge(dsem, 96)
            sync.dma_start(
                texc_b[:], scr1[None, :].partition_broadcast(128)
            ).then_inc(dsem, 16)  # cum 112
            # final output store (host sums the 128 partial pairs)
            sync.wait_ge(vv, VV_DONE)
            sync.dma_start(out_d[:], red[:, 0:2]).then_inc(dsem, 16)

        @block.scalar
        def _(scalar):
            scalar.wait_ge(dsem, 80)
            # w = exp(-s) compact, bounce through DRAM, broadcast-read
            scalar.activation(wcmp_s[:], scmp_s[:], Act.Exp, scale=-1.0).then_inc(
                asem, 1
            )
            scalar.wait_ge(asem, 1)  # drain ACT pipe before DMA reads wcmp
            scalar.dma_start(
                scr2.rearrange("(p b) -> p b", b=BCOL), wcmp_s[:]
            ).then_inc(dsem2, 16)
            scalar.wait_ge(dsem2, 16)
            scalar.dma_start(
                w_b[:], scr2[None, :].partition_broadcast(128)
            ).then_inc(dsem2, 16)  # cum 32
            # v_j = exp(s_j) for the epilogue
            scalar.activation(vj_s[:], sj_s[:], Act.Exp).then_inc(asem, 1)

        @block.vector
        def _(vector):
            n = 0

            def step(ins):
                nonlocal n
                n += 1
                ins.then_inc(vv, 1)

            vector.wait_ge(dsem, 80)
            # t'_i = t_i + 1e30 * (e_i == 0)   (compact, tiny)
            step(vector.tensor_scalar(
                out=tmp_s[:], in0=ecmp_s[:], scalar1=0.0, scalar2=1e30,
                op0=Alu.is_equal, op1=Alu.mult,
            ))
            vector.wait_ge(vv, n)
            step(vector.tensor_add(texc_s[:], tmp_s[:], tcmp_s[:]))
            assert n == VV_TEXC
            vector.wait_ge(dsem, 112)  # texc_b broadcast done
            for b in range(NBLK):
                # pass1: m = (t'_i < t_j); fused row reduce -> pair counts
                vector.wait_ge(vv, n)
                step(vector.tensor_scalar(
                    out=m_t[:], in0=texc_b[:],
                    scalar1=tj_s[:, b : b + 1], scalar2=None,
                    op0=Alu.is_lt, op1=Alu.add,
                    accum_out=crows[:, b : b + 1],
                ))
                if b == 0:
                    vector.wait_ge(dsem2, 32)  # w_b broadcast done
                # pass2: lrows[j] = sum_i m * exp(-s_i)
                vector.wait_ge(vv, n)
                step(vector.scalar_tensor_tensor(
                    out=junk[:], in0=m_t[:], scalar=0.0, in1=w_b[:],
                    op0=Alu.add, op1=Alu.mult,
                    accum_out=lrows[:, b : b + 1],
                ))
            assert n == VV_MAIN
            # epilogue: scale by exp(s_j), reduce to two scalars
            vector.wait_ge(asem, 2)
            vector.wait_ge(vv, n)
            step(vector.tensor_mul(lrows[:], lrows[:], vj_s[:]))
            vector.wait_ge(vv, n)
            step(vector.reduce_sum(out=red[:, 0:1], in_=lrows[:], axis=X))
            vector.wait_ge(vv, n)
            step(vector.reduce_sum(out=red[:, 1:2], in_=crows[:], axis=X))
            assert n == VV_DONE

    return nc


def _in_maps(preds, targets):
    t = np.ascontiguousarray(targets[:, 0], dtype=np.float32)
    e = np.ascontiguousarray(targets[:, 1], dtype=np.float32)
    s = np.ascontiguousarray(preds, dtype=np.float32).reshape(-1)
    shared = {
        "tcmp": np.ascontiguousarray(t.reshape(128, BCOL)),
        "ecmp": np.ascontiguousarray(e.reshape(128, BCOL)),
        "scmp": np.ascontiguousarray(s.reshape(128, BCOL)),
    }
    maps = []
    for c in range(NCORES):
        jsl = slice(c * JPC, (c + 1) * JPC)
        maps.append(
            dict(
                shared,
                tj=np.ascontiguousarray(t[jsl].reshape(NBLK, 128).T),
                sj=np.ascontiguousarray(s[jsl].reshape(NBLK, 128).T),
            )
        )
    return maps


def _combine(results):
    loss_sum = 0.0
    count = 0.0
    for r in results:
        part = np.asarray(r["out"], dtype=np.float64)
        loss_sum += part[:, 0].sum()
        count += part[:, 1].sum()
    return np.array(np.float32(loss_sum) / np.float32(max(count, 1.0)),
                    dtype=np.float32)


def kernel(preds, targets):
    from concourse.bass_utils import run_bass_kernel_spmd

    if "nc" not in _CACHE:
        _CACHE["nc"] = _build()
    nc = _CACHE["nc"]
    res = run_bass_kernel_spmd(nc, _in_maps(preds, targets), list(range(NCORES)))
    return _combine(res.results)


# revision 13
# speedup vs baseline: 2.4838x; 2.4838x over previous
"""Trainium2 Bass kernel for ExponentialConcordanceLoss.

Reference semantics (N = 8192):
    t = targets[:, 0]; e = targets[:, 1] != 0; s = preds
    mask[j, i] = (t[i] < t[j]) & e[i]            (all inputs finite)
    loss = sum_{j,i} mask * exp(s[j] - s[i]) / max(sum(mask), 1)

Factorization used on device:
    loss_sum = sum_j exp(s[j]) * (sum_i mask[j,i] * exp(-s[i]))
    count    = sum_{j,i} mask[j,i]

Sharding (v2, transposed): each core owns a 1024-wide slice of the
i-axis (8 blocks of 128 on the partition dim) and the full j-axis
(8192) on the free dim. The event filter folds into the compare via
t'_i = t_i + 1e30*(e_i==0). Per i-block:
  pass1 (DVE, fp32-compare -> bf16 mask, 2x mode):
      m_T[i, j] = (t_j > t'_i), fused row-reduce -> exact pair counts
  pass2 (TensorEngine): psum[j, :] += m_T_chunk.T @ [w_hi, w_lo]
      where w_hi + w_lo is a bf16 hi/lo split of exp(-s_i) (the split
      restores ~fp32 accuracy; ldweights requires 16-bit anyway).
The 64 psum column-pairs accumulate over the 8 i-blocks in one PSUM
bank. Epilogue: loss_rows = (hi+lo) * exp(s_j), reduce; host sums the
8x[128,2] partials and divides.
"""

import sys

if "/opt/trn_rl_repo" not in sys.path:
    sys.path.insert(0, "/opt/trn_rl_repo")

import numpy as np

N = 8192
NCORES = 8
IPC = N // NCORES      # i's per core
NBLK = IPC // 128      # i-blocks per core (8)
NCH = N // 128         # j chunks of 128 (64)

_CACHE = {}


def _build():
    """Trace the (SPMD) Bass program. Same program on every core; the
    per-core i-slice arrives via the tploc/eploc/sploc inputs."""
    import concourse.bass as bass
    import concourse.mybir as mybir

    f32 = mybir.dt.float32
    bf16 = mybir.dt.bfloat16
    Alu = mybir.AluOpType
    Act = mybir.ActivationFunctionType
    X = mybir.AxisListType.X

    nc = bass.Bass()

    tflat_d = nc.dram_tensor("tflat", [N], f32, kind="ExternalInput")
    tploc_d = nc.dram_tensor("tploc", [128, NBLK], f32, kind="ExternalInput")
    eploc_d = nc.dram_tensor("eploc", [128, NBLK], f32, kind="ExternalInput")
    sploc_d = nc.dram_tensor("sploc", [128, NBLK], f32, kind="ExternalInput")
    sjb_d = nc.dram_tensor("sjb", [128, NCH], f32, kind="ExternalInput")
    out_d = nc.dram_tensor("out", [128, 2], f32, kind="ExternalOutput")

    from contextlib import ExitStack

    with ExitStack() as ctx:
        en = ctx.enter_context
        tploc_s = en(nc.sbuf_tensor([128, NBLK], f32))
        eploc_s = en(nc.sbuf_tensor([128, NBLK], f32))
        sploc_s = en(nc.sbuf_tensor([128, NBLK], f32))
        sjb_s = en(nc.sbuf_tensor([128, NCH], f32))
        tmp8 = en(nc.sbuf_tensor([128, NBLK], f32))
        texc_loc = en(nc.sbuf_tensor([128, NBLK], f32))
        w_f32 = en(nc.sbuf_tensor([128, NBLK], f32))
        whi = en(nc.sbuf_tensor([128, NBLK], bf16))
        wlo_f = en(nc.sbuf_tensor([128, NBLK], f32))
        wpair = en(nc.sbuf_tensor([128, 2 * NBLK], bf16))
        vjb = en(nc.sbuf_tensor([128, NCH], f32))
        cntT = en(nc.sbuf_tensor([128, NBLK], f32))
        lrows = en(nc.sbuf_tensor([128, NCH], f32))
        red = en(nc.sbuf_tensor([128, 2], f32))
        tjb = en(nc.sbuf_tensor([128, N], f32))
        mA = en(nc.sbuf_tensor([128, N], bf16))
        mB = en(nc.sbuf_tensor([128, N], bf16))
        ptile = en(nc.psum_tensor([128, 2 * NCH], f32))
        dsem = en(nc.semaphore())
        dsem2 = en(nc.semaphore())
        outsem = en(nc.semaphore())
        asem = en(nc.semaphore())
        vv = en(nc.semaphore())
        pesem = en(nc.semaphore())
        block = en(nc.Block())
        mbufs = [mA, mB]

        # vv checkpoints
        VV_TEXC = 2          # t' ready
        VV_WPAIR = 6         # wpair ready
        VV_P1 = lambda k: VV_WPAIR + k + 1   # pass1 of i-block k done
        VV_DONE = VV_WPAIR + NBLK + 4        # epilogue done

        @block.sync
        def _(sync):
            sync.dma_start(tploc_s[:], tploc_d[:]).then_inc(dsem, 16)
            sync.dma_start(eploc_s[:], eploc_d[:]).then_inc(dsem, 16)
            sync.dma_start(sploc_s[:], sploc_d[:]).then_inc(dsem, 16)
            sync.dma_start(sjb_s[:], sjb_d[:]).then_inc(dsem, 16)
            # broadcast t over all partitions (the only big transfer)
            sync.dma_start(
                tjb[:], tflat_d[None, :].partition_broadcast(128)
            ).then_inc(dsem2, 16)
            sync.wait_ge(vv, VV_DONE)
            sync.dma_start(out_d[:], red[:, 0:2]).then_inc(outsem, 16)
            sync.wait_ge(outsem, 16)

        @block.scalar
        def _(scalar):
            scalar.wait_ge(dsem, 64)
            scalar.activation(w_f32[:], sploc_s[:], Act.Exp, scale=-1.0).then_inc(
                asem, 1
            )
            scalar.activation(vjb[:], sjb_s[:], Act.Exp).then_inc(asem, 1)

        @block.vector
        def _(vector):
            n = 0

            def step(ins):
                nonlocal n
                n += 1
                ins.then_inc(vv, 1)

            vector.wait_ge(dsem, 64)
            # t'_i = t_i + 1e30 * (e_i == 0)
            step(vector.tensor_scalar(
                out=tmp8[:], in0=eploc_s[:], scalar1=0.0, scalar2=1e30,
                op0=Alu.is_equal, op1=Alu.mult,
            ))
            vector.wait_ge(vv, n)
            step(vector.tensor_add(texc_loc[:], tmp8[:], tploc_s[:]))
            assert n == VV_TEXC
            # bf16 hi/lo split of w = exp(-s_i)
            vector.wait_ge(asem, 1)
            step(vector.tensor_copy(whi[:], w_f32[:]))
            vector.wait_ge(vv, n)
            step(vector.tensor_sub(wlo_f[:], w_f32[:], whi[:]))
            vector.wait_ge(vv, n)
            step(vector.tensor_copy(wpair[:, 0 : 2 * NBLK : 2], whi[:]))
            vector.wait_ge(vv, n)
            step(vector.tensor_copy(wpair[:, 1 : 2 * NBLK : 2], wlo_f[:]))
            assert n == VV_WPAIR
            vector.wait_ge(dsem2, 16)  # tjb broadcast done
            for k in range(NBLK):
                if k >= 2:
                    vector.wait_ge(pesem, k - 1)  # PE done with this buffer
                vector.wait_ge(vv, n)
                step(vector.tensor_scalar(
                    out=mbufs[k % 2][:], in0=tjb[:],
                    scalar1=texc_loc[:, k : k + 1], scalar2=None,
                    op0=Alu.is_gt, op1=Alu.add,
                    accum_out=cntT[:, k : k + 1],
                ))
                assert n == VV_P1(k)
            # epilogue
            vector.wait_ge(pesem, NBLK)
            step(vector.tensor_add(
                lrows[:], ptile[:, 0 : 2 * NCH : 2], ptile[:, 1 : 2 * NCH : 2]
            ))
            vector.wait_ge(asem, 2)
            vector.wait_ge(vv, n)
            step(vector.tensor_mul(lrows[:], lrows[:], vjb[:]))
            vector.wait_ge(vv, n)
            step(vector.reduce_sum(out=red[:, 0:1], in_=lrows[:], axis=X))
            vector.wait_ge(vv, n)
            step(vector.reduce_sum(out=red[:, 1:2], in_=cntT[:], axis=X))
            assert n == VV_DONE

        @block.tensor
        def _(tensor):
            tensor.wait_ge(vv, VV_WPAIR)
            for k in range(NBLK):
                tensor.wait_ge(vv, VV_P1(k))
                m = mbufs[k % 2]
                for c in range(NCH):
                    # start marks the whole 2KB zero-region pending-zero, so
                    # it must be issued exactly once per PSUM bank; the first
                    # write to each column then auto-zeroes.
                    ins = tensor.matmul(
                        ptile[:, 2 * c : 2 * c + 2],
                        m[:, 128 * c : 128 * (c + 1)],
                        wpair[:, 2 * k : 2 * k + 2],
                        start=(k == 0 and c == 0),
                        stop=(k == NBLK - 1 and c == NCH - 1),
                    )
                ins.then_inc(pesem, 1)

    return nc


def _in_maps(preds, targets):
    t = np.ascontiguousarray(targets[:, 0], dtype=np.float32)
    e = np.ascontiguousarray(targets[:, 1], dtype=np.float32)
    s = np.ascontiguousarray(preds, dtype=np.float32).reshape(-1)
    shared = {
        "tflat": t,
        "sjb": np.ascontiguousarray(s.reshape(NCH, 128).T),
    }
    maps = []
    for c in range(NCORES):
        isl = slice(c * IPC, (c + 1) * IPC)
        maps.append(
            dict(
                shared,
                tploc=np.ascontiguousarray(t[isl].reshape(NBLK, 128).T),
                eploc=np.ascontiguousarray(e[isl].reshape(NBLK, 128).T),
                sploc=np.ascontiguousarray(s[isl].reshape(NBLK, 128).T),
            )
        )
    return maps


def _combine(results):
    loss_sum = 0.0
    count = 0.0
    for r in results:
        part = np.asarray(r["out"], dtype=np.float64)
        loss_sum += part[:, 0].sum()
        count += part[:, 1].sum()
    return np.array(np.float32(loss_sum) / np.float32(max(count, 1.0)),
                    dtype=np.float32)


def kernel(preds, targets):
    from concourse.bass_utils import run_bass_kernel_spmd

    if "nc" not in _CACHE:
        _CACHE["nc"] = _build()
    nc = _CACHE["nc"]
    res = run_bass_kernel_spmd(nc, _in_maps(preds, targets), list(range(NCORES)))
    return _combine(res.results)


# revision 19
# speedup vs baseline: 5.0401x; 2.0292x over previous
"""Trainium2 Bass kernel for ExponentialConcordanceLoss.

Reference semantics (N = 8192):
    t = targets[:, 0]; e = targets[:, 1] != 0; s = preds
    mask[j, i] = (t[i] < t[j]) & e[i]            (all inputs finite)
    loss = sum_{j,i} mask * exp(s[j] - s[i]) / max(sum(mask), 1)

Factorization used on device:
    loss_sum = sum_j exp(s[j]) * (sum_i mask[j,i] * exp(-s[i]))
    count    = sum_{j,i} mask[j,i]

v3 layout: the i-axis keeps only event rows (non-events never fire the
mask), sorted by time; the j-axis is the full 8192 sorted by time.
Sorting is pure host-side layout prep - every compare/exp/product/
reduction still runs on device. For a 128-row i-block whose smallest
t' is v, every j with t_j <= v gives mask 0, so the block only needs
columns [jstart, 8192) where jstart = searchsorted(t_sorted, v) rounded
down to 128. Blocks are sorted by jstart and dealt round-robin into
"slots" of 8 (one block per core per slot), so the compiled program -
shared by all cores - has one static width per slot and the cores stay
perfectly balanced.

Per slot:
  pass1 (DVE, fp32 compare -> bf16 mask, 2x mode):
      m_T[i, j] = (t_j > t'_i) over [jstart, 8192), fused row-reduce
      gives exact pair counts
  pass2 (TensorEngine): psum[j, :] += m_T_chunk.T @ [w_hi, w_lo]
      (bf16 hi/lo split of exp(-s_i) keeps ~fp32 accuracy)
The t broadcast is split: DMA broadcast-reads the low half of the
sorted t row while GPSIMD partition-broadcasts the high half, tail
chunks first, so narrow (high-jstart) slots start almost immediately.
Epilogue: loss_rows = (hi+lo) * exp(s_j), reduce; the host sums the
8x[128,2] partials and divides.

The program is compiled per slot-width tuple (input-data metadata);
repeated calls with the same shape of data reuse the cache.
"""

import sys

if "/opt/trn_rl_repo" not in sys.path:
    sys.path.insert(0, "/opt/trn_rl_repo")

import numpy as np

N = 8192
NCORES = 8
NCH = N // 128         # j chunks of 128 (64)
CHUNKS = (0, 2048, 4096, 6144, 8192)  # broadcast chunk boundaries

_CACHE = {}


def _build(widths):
    """Trace the SPMD Bass program for the given per-slot widths
    (each a multiple of 128; slot q covers j in [N-width, N))."""
    import concourse.bass as bass
    import concourse.mybir as mybir

    f32 = mybir.dt.float32
    bf16 = mybir.dt.bfloat16
    Alu = mybir.AluOpType
    Act = mybir.ActivationFunctionType
    X = mybir.AxisListType.X

    nslots = len(widths)
    jstarts = [N - w for w in widths]

    nc = bass.Bass()

    tflat_d = nc.dram_tensor("tflat", [N], f32, kind="ExternalInput")
    tploc_d = nc.dram_tensor("tploc", [128, nslots], f32, kind="ExternalInput")
    eploc_d = nc.dram_tensor("eploc", [128, nslots], f32, kind="ExternalInput")
    sploc_d = nc.dram_tensor("sploc", [128, nslots], f32, kind="ExternalInput")
    sjb_d = nc.dram_tensor("sjb", [128, NCH], f32, kind="ExternalInput")
    out_d = nc.dram_tensor("out", [128, 2], f32, kind="ExternalOutput")

    from contextlib import ExitStack

    with ExitStack() as ctx:
        en = ctx.enter_context
        tploc_s = en(nc.sbuf_tensor([128, nslots], f32))
        eploc_s = en(nc.sbuf_tensor([128, nslots], f32))
        sploc_s = en(nc.sbuf_tensor([128, nslots], f32))
        sjb_s = en(nc.sbuf_tensor([128, NCH], f32))
        trow_s = en(nc.sbuf_tensor([1, N], f32))
        tmp8 = en(nc.sbuf_tensor([128, nslots], f32))
        texc_loc = en(nc.sbuf_tensor([128, nslots], f32))
        w_f32 = en(nc.sbuf_tensor([128, nslots], f32))
        whi = en(nc.sbuf_tensor([128, nslots], bf16))
        wlo_f = en(nc.sbuf_tensor([128, nslots], f32))
        wpair = en(nc.sbuf_tensor([128, 2 * nslots], bf16))
        vjb = en(nc.sbuf_tensor([128, NCH], f32))
        cntT = en(nc.sbuf_tensor([128, nslots], f32))
        lrows = en(nc.sbuf_tensor([128, NCH], f32))
        red = en(nc.sbuf_tensor([128, 2], f32))
        pef_s = en(nc.sbuf_tensor([128, 2 * NCH], f32))
        tjb = en(nc.sbuf_tensor([128, N], f32))
        mA = en(nc.sbuf_tensor([128, N], bf16))
        mB = en(nc.sbuf_tensor([128, N], bf16))
        ptile = en(nc.psum_tensor([128, 2 * NCH], f32))
        dsem = en(nc.semaphore())    # small loads
        dsem2 = en(nc.semaphore())   # DMA broadcast chunks
        dsem3 = en(nc.semaphore())   # trow load
        gsem = en(nc.semaphore())    # gpsimd broadcast chunks
        outsem = en(nc.semaphore())
        asem = en(nc.semaphore())
        vv = en(nc.semaphore())
        pesem = en(nc.semaphore())
        block = en(nc.Block())
        mbufs = [mA, mB]

        # broadcast chunk readiness: chunk 3 [6144:8192) -> gsem>=1,
        # chunk 2 [4096:6144) -> gsem>=2, chunk 1 [2048:4096) -> dsem2>=16,
        # chunk 0 [0:2048) -> dsem2>=32.
        def bcast_waits(eng, jstart):
            if jstart < CHUNKS[1]:
                eng.wait_ge(dsem2, 32)
            elif jstart < CHUNKS[2]:
                eng.wait_ge(dsem2, 16)
            if jstart < CHUNKS[3]:
                eng.wait_ge(gsem, 2)
            else:
                eng.wait_ge(gsem, 1)

        VV_WPAIR = 7
        VV_P1 = lambda q: VV_WPAIR + q + 1
        VV_DONE = VV_WPAIR + nslots + 5

        @block.sync
        def _(sync):
            sync.dma_start(tploc_s[:], tploc_d[:]).then_inc(dsem, 16)
            sync.dma_start(eploc_s[:], eploc_d[:]).then_inc(dsem, 16)
            sync.dma_start(sploc_s[:], sploc_d[:]).then_inc(dsem, 16)
            sync.dma_start(sjb_s[:], sjb_d[:]).then_inc(dsem, 16)
            sync.dma_start(trow_s[:], tflat_d[None, :]).then_inc(dsem3, 16)
            # DMA broadcast of the low half, tail chunk first
            sync.dma_start(
                tjb[:, CHUNKS[1] : CHUNKS[2]],
                tflat_d[None, CHUNKS[1] : CHUNKS[2]].partition_broadcast(128),
            ).then_inc(dsem2, 16)
            sync.wait_ge(dsem2, 16)  # keep dsem2 increments deterministic
            sync.dma_start(
                tjb[:, CHUNKS[0] : CHUNKS[1]],
                tflat_d[None, CHUNKS[0] : CHUNKS[1]].partition_broadcast(128),
            ).then_inc(dsem2, 16)
            sync.wait_ge(vv, VV_DONE)
            sync.dma_start(out_d[:], red[:, 0:2]).then_inc(outsem, 16)
            sync.wait_ge(outsem, 16)

        @block.gpsimd
        def _(gpsimd):
            from concourse import library_config

            # partition-broadcast of the high half, tail chunk first
            gpsimd.load_library(library_config.mlp)
            gpsimd.wait_ge(dsem3, 16)
            gpsimd.partition_broadcast(
                tjb[:, CHUNKS[3] : CHUNKS[4]], trow_s[0:1, CHUNKS[3] : CHUNKS[4]]
            ).then_inc(gsem, 1)
            gpsimd.partition_broadcast(
                tjb[:, CHUNKS[2] : CHUNKS[3]], trow_s[0:1, CHUNKS[2] : CHUNKS[3]]
            ).then_inc(gsem, 1)

        @block.scalar
        def _(scalar):
            scalar.wait_ge(dsem, 64)
            scalar.activation(w_f32[:], sploc_s[:], Act.Exp, scale=-1.0).then_inc(
                asem, 1
            )
            scalar.activation(vjb[:], sjb_s[:], Act.Exp).then_inc(asem, 1)

        @block.vector
        def _(vector):
            n = 0

            def step(ins):
                nonlocal n
                n += 1
                ins.then_inc(vv, 1)

            vector.wait_ge(dsem, 64)
            # t'_i = t_i + 1e30 * (e_i == 0)
            step(vector.tensor_scalar(
                out=tmp8[:], in0=eploc_s[:], scalar1=0.0, scalar2=1e30,
                op0=Alu.is_equal, op1=Alu.mult,
            ))
            vector.wait_ge(vv, n)
            step(vector.tensor_add(texc_loc[:], tmp8[:], tploc_s[:]))
            # bf16 hi/lo split of w = exp(-s_i)
            vector.wait_ge(asem, 1)
            step(vector.tensor_copy(whi[:], w_f32[:]))
            vector.wait_ge(vv, n)
            step(vector.tensor_sub(wlo_f[:], w_f32[:], whi[:]))
            vector.wait_ge(vv, n)
            step(vector.tensor_copy(wpair[:, 0 : 2 * nslots : 2], whi[:]))
            vector.wait_ge(vv, n)
            step(vector.tensor_copy(wpair[:, 1 : 2 * nslots : 2], wlo_f[:]))
            vector.wait_ge(vv, n)
            step(vector.memset(ptile[:], 0.0))
            assert n == VV_WPAIR
            for q in range(nslots):
                js = jstarts[q]
                bcast_waits(vector, js)
                if q >= 2:
                    vector.wait_ge(pesem, q - 1)  # PE done with this buffer
                vector.wait_ge(vv, n)
                step(vector.tensor_scalar(
                    out=mbufs[q % 2][:, js:N], in0=tjb[:, js:N],
                    scalar1=texc_loc[:, q : q + 1], scalar2=None,
                    op0=Alu.is_gt, op1=Alu.add,
                    accum_out=cntT[:, q : q + 1],
                ))
                assert n == VV_P1(q)
            # epilogue (only one PSUM operand allowed per DVE op)
            vector.wait_ge(pesem, nslots)
            step(vector.tensor_copy(pef_s[:], ptile[:]))
            vector.wait_ge(vv, n)
            step(vector.tensor_add(
                lrows[:], pef_s[:, 0 : 2 * NCH : 2], pef_s[:, 1 : 2 * NCH : 2]
            ))
            vector.wait_ge(asem, 2)
            vector.wait_ge(vv, n)
            step(vector.tensor_mul(lrows[:], lrows[:], vjb[:]))
            vector.wait_ge(vv, n)
            step(vector.reduce_sum(out=red[:, 0:1], in_=lrows[:], axis=X))
            vector.wait_ge(vv, n)
            step(vector.reduce_sum(out=red[:, 1:2], in_=cntT[:], axis=X))
            assert n == VV_DONE

        @block.tensor
        def _(tensor):
            tensor.wait_ge(vv, VV_WPAIR)
            started = set()
            for q in range(nslots):
                tensor.wait_ge(vv, VV_P1(q))
                m = mbufs[q % 2]
                for c in range(jstarts[q] // 128, NCH):
                    # 'start' marks a whole 2KB zero-region pending-zero, so
                    # issue it once per region (4 regions in the [128, 128]
                    # f32 ptile row: 512 f32 = one region). First touch of
                    # each column then auto-zeroes.
                    region = (2 * c * 4) // 2048  # byte region index
                    first = region not in started
                    started.add(region)
                    ins = tensor.matmul(
                        ptile[:, 2 * c : 2 * c + 2],
                        m[:, 128 * c : 128 * (c + 1)],
                        wpair[:, 2 * q : 2 * q + 2],
                        start=first,
                        stop=(q == nslots - 1 and c == NCH - 1),
                        skip_group_check=True,
                    )
                ins.then_inc(pesem, 1)

    return nc


def _plan(preds, targets):
    """Host-side layout prep: sort, block, and slot the work."""
    t = np.ascontiguousarray(targets[:, 0], dtype=np.float32)
    e = np.ascontiguousarray(targets[:, 1], dtype=np.float32)
    s = np.ascontiguousarray(preds, dtype=np.float32).reshape(-1)

    orderj = np.argsort(t, kind="stable")
    t_j = t[orderj]
    s_j = s[orderj]

    ev = np.flatnonzero(e != 0.0)
    if len(ev) == 0:
        return None
    ev = ev[np.argsort(t[ev], kind="stable")]
    nblocks = -(-len(ev) // 128)
    nblocks_pad = -(-nblocks // NCORES) * NCORES

    # per-block (t, e, s) rows and jstart
    bt = np.zeros((nblocks_pad, 128), np.float32)
    be = np.zeros((nblocks_pad, 128), np.float32)
    bs = np.zeros((nblocks_pad, 128), np.float32)
    jstart = np.full(nblocks_pad, N, np.int64)
    for b in range(nblocks):
        idx = ev[b * 128 : (b + 1) * 128]
        k = len(idx)
        bt[b, :k] = t[idx]
        be[b, :k] = 1.0
        bs[b, :k] = s[idx]
        js = int(np.searchsorted(t_j, t[idx[0]], side="right"))
        jstart[b] = (js // 128) * 128

    # deal blocks (sorted by jstart desc) into slots of NCORES
    order_b = np.argsort(-jstart, kind="stable")
    nslots = nblocks_pad // NCORES
    widths = []
    slot_blocks = []
    for q in range(nslots):
        grp = order_b[q * NCORES : (q + 1) * NCORES]
        js = int(jstart[grp].min())
        w = max(128, N - js)
        widths.append(w)
        slot_blocks.append(grp)

    maps = []
    shared = {
        "tflat": t_j,
        "sjb": np.ascontiguousarray(s_j.reshape(NCH, 128).T),
    }
    for c in range(NCORES):
        tploc = np.zeros((128, nslots), np.float32)
        eploc = np.zeros((128, nslots), np.float32)
        sploc = np.zeros((128, nslots), np.float32)
        for q in range(nslots):
            b = slot_blocks[q][c]
            tploc[:, q] = bt[b]
            eploc[:, q] = be[b]
            sploc[:, q] = bs[b]
        maps.append(dict(shared, tploc=tploc, eploc=eploc, sploc=sploc))
    return tuple(widths), maps


def _combine(results):
    loss_sum = 0.0
    count = 0.0
    for r in results:
        part = np.asarray(r["out"], dtype=np.float64)
        loss_sum += part[:, 0].sum()
        count += part[:, 1].sum()
    return np.array(np.float32(loss_sum) / np.float32(max(count, 1.0)),
                    dtype=np.float32)


def kernel(preds, targets):
    from concourse.bass_utils import run_bass_kernel_spmd

    plan = _plan(preds, targets)
    if plan is None:
        return np.array(0.0, dtype=np.float32)
    widths, maps = plan
    if widths not in _CACHE:
        _CACHE[widths] = _build(widths)
    nc = _CACHE[widths]
    res = run_bass_kernel_spmd(nc, maps, list(range(NCORES)))
    return _combine(res.results)
